# revision 1
# baseline (speedup 1.0000x reference)
"""Trainium2 Bass kernel for DLLinearZeroDiagonal:
    y = x @ W.T + bias,  W = zero-diagonal 4096x4096 with strict triangles
    packed row-major in upper_w / lower_w.

Strategy (8 NeuronCores):
  - 2-way shard over output dim (o) x 4-way shard over batch (b).
  - Host reconstructs the dense weight (sanctioned by the sharding hint:
    "replicate the reconstructed weight") and lays out W^T / x^T shards in
    the tile order the device DMAs want.  All FLOPs + bias happen on device.
  - Per core: resident x^T shard (16 MB SBUF), stream W^T slabs once,
    1024 accumulating fp32r matmuls (128x128 @ 128x512), bias add on DVE,
    outputs written as y^T shard and untransposed on host.
"""

import numpy as np

N = 4096            # in/out feature dim and batch
RO, RB = 2, 4       # shard ways over output-dim / batch
OC = N // RO        # 2048 output cols per core
BC = N // RB        # 1024 batch rows per core
NW = OC // 128      # 16 stationary o-blocks per core
NT = N // 128       # 32 contraction tiles
NN = BC // 512      # 2 moving b-tiles per core

_PROGRAM = None


JC = 4              # j-chunks (chunked variant)
NTC = NT // JC      # t-tiles per chunk


def _build_program(reps=None, variant="resident2"):
    # HW-measured (R=257 on-device repeat loops, median-differenced):
    #   resident: ~351 us/core   chunked: ~430 us/core   resident2: ~314 us/core
    # resident2 = resident with the x^T loads moved to the ACT HWDGE ring so
    # the first weight slab (SP ring) isn't FIFO-queued behind the 16 MB x^T
    # stream; the chunked variant's 512B-row weight mini-slab DMAs cost more
    # than the startup stall they eliminate.
    if variant == "chunked":
        return _build_program_chunked(reps)
    if variant == "resident2":
        return _build_program_resident2(reps)
    return _build_program_resident(reps)


def _build_program_chunked(reps=None):
    """Stream x^T in j-chunks with an SBUF accumulator: no 16 MB startup
    stall (first matmul only waits on a 2 MB chunk + 0.5 MB weight slab)."""
    import concourse.bacc as bacc
    import concourse.bass as bass
    import concourse.tile as tile
    from concourse import mybir
    from contextlib import ExitStack, nullcontext

    F32 = mybir.dt.float32
    F32R = mybir.dt.float32r

    nc = bacc.Bacc("TRN2", target_bir_lowering=False, debug=False)
    xt = nc.dram_tensor("xt", [NT, 128, BC], F32R, kind="ExternalInput")
    wt = nc.dram_tensor("wt", [NW, 128, NT, 128], F32R, kind="ExternalInput")
    bias = nc.dram_tensor("bias", [128, NW], F32, kind="ExternalInput")
    yt = nc.dram_tensor("yt", [OC, BC], F32, kind="ExternalOutput")

    with tile.TileContext(nc) as tc, ExitStack() as ctx:
        xcp = ctx.enter_context(tc.tile_pool(name="xcp", bufs=2))
        wcp = ctx.enter_context(tc.tile_pool(name="wcp", bufs=3))
        acp = ctx.enter_context(tc.tile_pool(name="acp", bufs=1))
        bp = ctx.enter_context(tc.tile_pool(name="bp", bufs=1))
        pp = ctx.enter_context(tc.tile_pool(name="pp", bufs=8, space="PSUM"))

        loop = tc.For_i(0, reps, 1) if reps is not None else nullcontext()
        with loop:
            bias_sb = bp.tile([128, NW], F32)
            nc.sync.dma_start(bias_sb[:], bass.AP(bias, 0, [[NW, 128], [1, NW]]))
            # accumulator: slice (w, n) at cols (w*NN+n)*512
            accum = acp.tile([128, NW * NN * 512], F32)

            for jc in range(JC):
                xch = xcp.tile([128, NTC * BC], F32R)
                nc.sync.dma_start(
                    xch[:],
                    bass.AP(xt, jc * NTC * 128 * BC,
                            [[BC, 128], [128 * BC, NTC], [1, BC]]),
                )
                for w in range(NW):
                    slab = wcp.tile([128, NTC * 128], F32R)
                    nc.sync.dma_start(
                        slab[:],
                        bass.AP(wt, w * 128 * NT * 128 + jc * NTC * 128,
                                [[NT * 128, 128], [128, NTC], [1, 128]]),
                    )
                    psums = [pp.tile([128, 512], F32, name=f"ps{n}", tag="ps")
                             for n in range(NN)]
                    for tc_i in range(NTC):
                        lhsT = slab[:, tc_i * 128:(tc_i + 1) * 128]
                        for n in range(NN):
                            nc.tensor.matmul(
                                psums[n][:],
                                lhsT,
                                xch[:, tc_i * BC + n * 512: tc_i * BC + n * 512 + 512],
                                start=(tc_i == 0),
                                stop=(tc_i == NTC - 1),
                            )
                    for n in range(NN):
                        acc = accum[:, (w * NN + n) * 512:(w * NN + n + 1) * 512]
                        if jc == 0:
                            nc.vector.tensor_scalar_add(acc, psums[n][:],
                                                        bias_sb[:, w:w + 1])
                        else:
                            nc.vector.tensor_add(acc, acc, psums[n][:])
                        if jc == JC - 1:
                            nc.scalar.dma_start(
                                bass.AP(yt, w * 128 * BC + n * 512,
                                        [[BC, 128], [1, 512]]),
                                acc,
                            )
    nc.compile()
    return nc


def _build_program_resident(reps=None):
    import concourse.bacc as bacc
    import concourse.bass as bass
    import concourse.tile as tile
    from concourse import mybir
    from contextlib import ExitStack, nullcontext

    F32 = mybir.dt.float32
    F32R = mybir.dt.float32r

    nc = bacc.Bacc("TRN2", target_bir_lowering=False, debug=False)
    # host-tiled layouts (see _shard_inputs):
    #   xt[t, p, b]     = x[b0+b, 128t+p]
    #   wt[w, p, t, o'] = W[o0+128w+o', 128t+p]
    #   bias2[p, w]     = bias[o0+128w+p]
    xt = nc.dram_tensor("xt", [NT, 128, BC], F32R, kind="ExternalInput")
    wt = nc.dram_tensor("wt", [NW, 128, NT, 128], F32R, kind="ExternalInput")
    bias = nc.dram_tensor("bias", [128, NW], F32, kind="ExternalInput")
    yt = nc.dram_tensor("yt", [OC, BC], F32, kind="ExternalOutput")

    with tile.TileContext(nc) as tc, ExitStack() as ctx:
        xtp = ctx.enter_context(tc.tile_pool(name="xtp", bufs=1))
        wtp = ctx.enter_context(tc.tile_pool(name="wtp", bufs=2))
        bp = ctx.enter_context(tc.tile_pool(name="bp", bufs=1))
        op = ctx.enter_context(tc.tile_pool(name="op", bufs=4))
        pp = ctx.enter_context(tc.tile_pool(name="pp", bufs=8, space="PSUM"))

        loop = tc.For_i(0, reps, 1) if reps is not None else nullcontext()
        with loop:
            # resident x^T shard: [128, NT*BC] ; column block t holds j=128t+p
            xt_res = xtp.tile([128, NT * BC], F32R)
            for t in range(NT):
                nc.sync.dma_start(
                    xt_res[:, t * BC:(t + 1) * BC],
                    bass.AP(xt, t * 128 * BC, [[BC, 128], [1, BC]]),
                )
            bias_sb = bp.tile([128, NW], F32)
            nc.sync.dma_start(bias_sb[:], bass.AP(bias, 0, [[NW, 128], [1, NW]]))

            for w in range(NW):
                # stationary slab for o-block w: [128 (j in t), NT*128 (t, o')]
                slab = wtp.tile([128, NT * 128], F32R)
                nc.sync.dma_start(
                    slab[:],
                    bass.AP(wt, w * 128 * NT * 128,
                            [[NT * 128, 128], [1, NT * 128]]),
                )
                psums = [pp.tile([128, 512], F32, name=f"ps{n}", tag="ps")
                         for n in range(NN)]
                for t in range(NT):
                    lhsT = slab[:, t * 128:(t + 1) * 128]
                    for n in range(NN):
                        nc.tensor.matmul(
                            psums[n][:],
                            lhsT,
                            xt_res[:, t * BC + n * 512: t * BC + n * 512 + 512],
                            start=(t == 0),
                            stop=(t == NT - 1),
                        )
                for n in range(NN):
                    ot = op.tile([128, 512], F32)
                    nc.vector.tensor_scalar_add(ot[:], psums[n][:],
                                                bias_sb[:, w:w + 1])
                    nc.scalar.dma_start(
                        bass.AP(yt, w * 128 * BC + n * 512, [[BC, 128], [1, 512]]),
                        ot[:],
                    )
    nc.compile()
    return nc


def _get_program():
    global _PROGRAM
    if _PROGRAM is None:
        _PROGRAM = _build_program()
    return _PROGRAM


def _reconstruct_wt(upper_w: np.ndarray, lower_w: np.ndarray) -> np.ndarray:
    """Dense W [o, j] from the packed strict triangles (row-major fill)."""
    W = np.zeros((N, N), dtype=np.float32)
    iu = np.triu_indices(N, k=1)
    il = np.tril_indices(N, k=-1)
    W[iu] = upper_w
    W[il] = lower_w
    return W


def _shard_inputs(x, upper_w, lower_w, bias):
    x = np.asarray(x, dtype=np.float32)
    upper_w = np.asarray(upper_w, dtype=np.float32)
    lower_w = np.asarray(lower_w, dtype=np.float32)
    bias = np.asarray(bias, dtype=np.float32)

    W = _reconstruct_wt(upper_w, lower_w)

    wt_shards = []
    bias_shards = []
    for ob in range(RO):
        Ws = W[ob * OC:(ob + 1) * OC, :]                       # [OC o, N j]
        # wt[w, p, t, o'] = Ws[128w+o', 128t+p]
        wt = np.ascontiguousarray(
            Ws.T.reshape(NT, 128, NW, 128).transpose(2, 1, 0, 3)
        )
        wt_shards.append(wt)
        bias_shards.append(
            np.ascontiguousarray(bias[ob * OC:(ob + 1) * OC].reshape(NW, 128).T)
        )

    xt_shards = []
    for bb in range(RB):
        xs = x[bb * BC:(bb + 1) * BC, :]                       # [BC b, N j]
        xt_shards.append(np.ascontiguousarray(xs.T.reshape(NT, 128, BC)))

    in_maps = []
    for c in range(8):
        ob, bb = c // RB, c % RB
        in_maps.append({
            "xt": xt_shards[bb],
            "wt": wt_shards[ob],
            "bias": bias_shards[ob],
        })
    return in_maps


def _assemble(results) -> np.ndarray:
    y = np.empty((N, N), dtype=np.float32)
    for c in range(8):
        ob, bb = c // RB, c % RB
        y[bb * BC:(bb + 1) * BC, ob * OC:(ob + 1) * OC] = results[c]["yt"].T
    return y


def kernel(x, upper_w, lower_w, bias):
    from concourse import bass_utils

    nc = _get_program()
    in_maps = _shard_inputs(x, upper_w, lower_w, bias)
    res = bass_utils.run_bass_kernel_spmd(nc, in_maps, core_ids=list(range(8)))
    return _assemble(res.results)


def _build_program_resident2(reps=None):
    import concourse.bacc as bacc
    import concourse.bass as bass
    import concourse.tile as tile
    from concourse import mybir
    from contextlib import ExitStack, nullcontext

    F32 = mybir.dt.float32
    F32R = mybir.dt.float32r

    nc = bacc.Bacc("TRN2", target_bir_lowering=False, debug=False)
    # host-tiled layouts (see _shard_inputs):
    #   xt[t, p, b]     = x[b0+b, 128t+p]
    #   wt[w, p, t, o'] = W[o0+128w+o', 128t+p]
    #   bias2[p, w]     = bias[o0+128w+p]
    xt = nc.dram_tensor("xt", [NT, 128, BC], F32R, kind="ExternalInput")
    wt = nc.dram_tensor("wt", [NW, 128, NT, 128], F32R, kind="ExternalInput")
    bias = nc.dram_tensor("bias", [128, NW], F32, kind="ExternalInput")
    yt = nc.dram_tensor("yt", [OC, BC], F32, kind="ExternalOutput")

    with tile.TileContext(nc) as tc, ExitStack() as ctx:
        xtp = ctx.enter_context(tc.tile_pool(name="xtp", bufs=1))
        wtp = ctx.enter_context(tc.tile_pool(name="wtp", bufs=2))
        bp = ctx.enter_context(tc.tile_pool(name="bp", bufs=1))
        op = ctx.enter_context(tc.tile_pool(name="op", bufs=4))
        pp = ctx.enter_context(tc.tile_pool(name="pp", bufs=8, space="PSUM"))

        loop = tc.For_i(0, reps, 1) if reps is not None else nullcontext()
        with loop:
            # resident x^T shard: [128, NT*BC] ; column block t holds j=128t+p
            xt_res = xtp.tile([128, NT * BC], F32R)
            for t in range(NT):
                nc.scalar.dma_start(
                    xt_res[:, t * BC:(t + 1) * BC],
                    bass.AP(xt, t * 128 * BC, [[BC, 128], [1, BC]]),
                )
            bias_sb = bp.tile([128, NW], F32)
            nc.sync.dma_start(bias_sb[:], bass.AP(bias, 0, [[NW, 128], [1, NW]]))

            for w in range(NW):
                # stationary slab for o-block w: [128 (j in t), NT*128 (t, o')]
                slab = wtp.tile([128, NT * 128], F32R)
                nc.sync.dma_start(
                    slab[:],
                    bass.AP(wt, w * 128 * NT * 128,
                            [[NT * 128, 128], [1, NT * 128]]),
                )
                psums = [pp.tile([128, 512], F32, name=f"ps{n}", tag="ps")
                         for n in range(NN)]
                for t in range(NT):
                    lhsT = slab[:, t * 128:(t + 1) * 128]
                    for n in range(NN):
                        nc.tensor.matmul(
                            psums[n][:],
                            lhsT,
                            xt_res[:, t * BC + n * 512: t * BC + n * 512 + 512],
                            start=(t == 0),
                            stop=(t == NT - 1),
                        )
                for n in range(NN):
                    ot = op.tile([128, 512], F32)
                    nc.vector.tensor_scalar_add(ot[:], psums[n][:],
                                                bias_sb[:, w:w + 1])
                    nc.scalar.dma_start(
                        bass.AP(yt, w * 128 * BC + n * 512, [[BC, 128], [1, 512]]),
                        ot[:],
                    )
    nc.compile()
    return nc


def _get_program():
    global _PROGRAM
    if _PROGRAM is None:
        _PROGRAM = _build_program()
    return _PROGRAM


def _reconstruct_wt(upper_w: np.ndarray, lower_w: np.ndarray) -> np.ndarray:
    """Dense W [o, j] from the packed strict triangles (row-major fill)."""
    W = np.zeros((N, N), dtype=np.float32)
    iu = np.triu_indices(N, k=1)
    il = np.tril_indices(N, k=-1)
    W[iu] = upper_w
    W[il] = lower_w
    return W


def _shard_inputs(x, upper_w, lower_w, bias):
    x = np.asarray(x, dtype=np.float32)
    upper_w = np.asarray(upper_w, dtype=np.float32)
    lower_w = np.asarray(lower_w, dtype=np.float32)
    bias = np.asarray(bias, dtype=np.float32)

    W = _reconstruct_wt(upper_w, lower_w)

    wt_shards = []
    bias_shards = []
    for ob in range(RO):
        Ws = W[ob * OC:(ob + 1) * OC, :]                       # [OC o, N j]
        # wt[w, p, t, o'] = Ws[128w+o', 128t+p]
        wt = np.ascontiguousarray(
            Ws.T.reshape(NT, 128, NW, 128).transpose(2, 1, 0, 3)
        )
        wt_shards.append(wt)
        bias_shards.append(
            np.ascontiguousarray(bias[ob * OC:(ob + 1) * OC].reshape(NW, 128).T)
        )

    xt_shards = []
    for bb in range(RB):
        xs = x[bb * BC:(bb + 1) * BC, :]                       # [BC b, N j]
        xt_shards.append(np.ascontiguousarray(xs.T.reshape(NT, 128, BC)))

    in_maps = []
    for c in range(8):
        ob, bb = c // RB, c % RB
        in_maps.append({
            "xt": xt_shards[bb],
            "wt": wt_shards[ob],
            "bias": bias_shards[ob],
        })
    return in_maps


def _assemble(results) -> np.ndarray:
    y = np.empty((N, N), dtype=np.float32)
    for c in range(8):
        ob, bb = c // RB, c % RB
        y[bb * BC:(bb + 1) * BC, ob * OC:(ob + 1) * OC] = results[c]["yt"].T
    return y


def kernel(x, upper_w, lower_w, bias):
    from concourse import bass_utils

    nc = _get_program()
    in_maps = _shard_inputs(x, upper_w, lower_w, bias)
    res = bass_utils.run_bass_kernel_spmd(nc, in_maps, core_ids=list(range(8)))
    return _assemble(res.results)



# revision 2
# speedup vs baseline: 1.1365x; 1.1365x over previous
"""Trainium2 Bass kernel for DLLinearZeroDiagonal:
    y = x @ W.T + bias,  W = zero-diagonal 4096x4096 with strict triangles
    packed row-major in upper_w / lower_w.

Strategy (8 NeuronCores):
  - 2-way shard over output dim (o) x 4-way shard over batch (b).
  - Host reconstructs the dense weight (sanctioned by the sharding hint:
    "replicate the reconstructed weight"), lays out W^T / x^T shards in
    the tile order the device DMAs want, and casts to bf16 (tolerance is
    2e-2; bf16 end-to-end error is ~5e-3).  All FLOPs + bias happen on
    device in fp32 PSUM.
  - Per core: resident x^T shard (8 MB SBUF bf16), stream W^T slabs
    (16 MB bf16), 1024 accumulating bf16 matmuls (128x128 @ 128x512),
    bias add on DVE writing bf16 outputs, untransposed + upcast on host.
  - bf16 vs the old fp32r: half the HBM bytes, and LDWEIGHTS becomes a
    separate instruction the PE pulls ahead of in-flight matmuls (fp32r
    forces a serial in-matmul weight load), so the PE streams at
    ~N cycles per matmul instead of ~N + 128.
"""

import numpy as np

N = 4096            # in/out feature dim and batch
RO, RB = 2, 4       # shard ways over output-dim / batch
OC = N // RO        # 2048 output cols per core
BC = N // RB        # 1024 batch rows per core
NW = OC // 128      # 16 stationary o-blocks per core
NT = N // 128       # 32 contraction tiles
NN = BC // 512      # 2 moving b-tiles per core

_PROGRAM = None


def _np_bf16():
    import ml_dtypes

    return np.dtype(ml_dtypes.bfloat16)


def _build_program(reps=None):
    import concourse.bacc as bacc
    import concourse.bass as bass
    import concourse.tile as tile
    from concourse import mybir
    from contextlib import ExitStack, nullcontext

    F32 = mybir.dt.float32
    BF16 = mybir.dt.bfloat16

    nc = bacc.Bacc("TRN2", target_bir_lowering=False, debug=False)
    # host-tiled layouts (see _shard_inputs):
    #   xt[t, p, b]     = x[b0+b, 128t+p]        (bf16)
    #   wt[w, p, t, o'] = W[o0+128w+o', 128t+p]  (bf16)
    #   bias2[p, w]     = bias[o0+128w+p]        (f32)
    xt = nc.dram_tensor("xt", [NT, 128, BC], BF16, kind="ExternalInput")
    wt = nc.dram_tensor("wt", [NW, 128, NT, 128], BF16, kind="ExternalInput")
    bias = nc.dram_tensor("bias", [128, NW], F32, kind="ExternalInput")
    yt = nc.dram_tensor("yt", [OC, BC], BF16, kind="ExternalOutput")

    with tile.TileContext(nc) as tc, ExitStack() as ctx:
        xtp = ctx.enter_context(tc.tile_pool(name="xtp", bufs=1))
        wtp = ctx.enter_context(tc.tile_pool(name="wtp", bufs=2))
        bp = ctx.enter_context(tc.tile_pool(name="bp", bufs=1))
        op = ctx.enter_context(tc.tile_pool(name="op", bufs=4))
        pp = ctx.enter_context(tc.tile_pool(name="pp", bufs=8, space="PSUM"))

        loop = tc.For_i(0, reps, 1) if reps is not None else nullcontext()
        with loop:
            # resident x^T shard: [128, NT*BC] ; column block t holds j=128t+p
            # (ACT HWDGE ring so the first weight slab on the SP ring isn't
            # queued behind the 8 MB x^T stream)
            xt_res = xtp.tile([128, NT * BC], BF16)
            for t in range(NT):
                nc.scalar.dma_start(
                    xt_res[:, t * BC:(t + 1) * BC],
                    bass.AP(xt, t * 128 * BC, [[BC, 128], [1, BC]]),
                )
            bias_sb = bp.tile([128, NW], F32)
            nc.sync.dma_start(bias_sb[:], bass.AP(bias, 0, [[NW, 128], [1, NW]]))

            for w in range(NW):
                # stationary slab for o-block w: [128 (j in t), NT*128 (t, o')]
                slab = wtp.tile([128, NT * 128], BF16)
                nc.sync.dma_start(
                    slab[:],
                    bass.AP(wt, w * 128 * NT * 128,
                            [[NT * 128, 128], [1, NT * 128]]),
                )
                psums = [pp.tile([128, 512], F32, name=f"ps{n}", tag="ps")
                         for n in range(NN)]
                for t in range(NT):
                    lhsT = slab[:, t * 128:(t + 1) * 128]
                    for n in range(NN):
                        nc.tensor.matmul(
                            psums[n][:],
                            lhsT,
                            xt_res[:, t * BC + n * 512: t * BC + n * 512 + 512],
                            start=(t == 0),
                            stop=(t == NT - 1),
                        )
                for n in range(NN):
                    ot = op.tile([128, 512], BF16)
                    nc.vector.tensor_scalar_add(ot[:], psums[n][:],
                                                bias_sb[:, w:w + 1])
                    nc.scalar.dma_start(
                        bass.AP(yt, w * 128 * BC + n * 512, [[BC, 128], [1, 512]]),
                        ot[:],
                    )
    nc.compile()
    return nc


def _get_program():
    global _PROGRAM
    if _PROGRAM is None:
        _PROGRAM = _build_program()
    return _PROGRAM


def _reconstruct_wt(upper_w: np.ndarray, lower_w: np.ndarray) -> np.ndarray:
    """Dense W [o, j] from the packed strict triangles (row-major fill)."""
    W = np.zeros((N, N), dtype=np.float32)
    iu = np.triu_indices(N, k=1)
    il = np.tril_indices(N, k=-1)
    W[iu] = upper_w
    W[il] = lower_w
    return W


def _shard_inputs(x, upper_w, lower_w, bias):
    bf16 = _np_bf16()
    x = np.asarray(x, dtype=np.float32)
    upper_w = np.asarray(upper_w, dtype=np.float32)
    lower_w = np.asarray(lower_w, dtype=np.float32)
    bias = np.asarray(bias, dtype=np.float32)

    W = _reconstruct_wt(upper_w, lower_w)

    wt_shards = []
    bias_shards = []
    for ob in range(RO):
        Ws = W[ob * OC:(ob + 1) * OC, :]                       # [OC o, N j]
        # wt[w, p, t, o'] = Ws[128w+o', 128t+p]
        wt = np.ascontiguousarray(
            Ws.T.reshape(NT, 128, NW, 128).transpose(2, 1, 0, 3).astype(bf16)
        )
        wt_shards.append(wt)
        bias_shards.append(
            np.ascontiguousarray(bias[ob * OC:(ob + 1) * OC].reshape(NW, 128).T)
        )

    xt_shards = []
    for bb in range(RB):
        xs = x[bb * BC:(bb + 1) * BC, :]                       # [BC b, N j]
        xt_shards.append(
            np.ascontiguousarray(xs.T.reshape(NT, 128, BC).astype(bf16))
        )

    in_maps = []
    for c in range(8):
        ob, bb = c // RB, c % RB
        in_maps.append({
            "xt": xt_shards[bb],
            "wt": wt_shards[ob],
            "bias": bias_shards[ob],
        })
    return in_maps


def _assemble(results) -> np.ndarray:
    y = np.empty((N, N), dtype=np.float32)
    for c in range(8):
        ob, bb = c // RB, c % RB
        y[bb * BC:(bb + 1) * BC, ob * OC:(ob + 1) * OC] = (
            results[c]["yt"].astype(np.float32).T
        )
    return y


def kernel(x, upper_w, lower_w, bias):
    from concourse import bass_utils

    nc = _get_program()
    in_maps = _shard_inputs(x, upper_w, lower_w, bias)
    res = bass_utils.run_bass_kernel_spmd(nc, in_maps, core_ids=list(range(8)))
    return _assemble(res.results)


# revision 3
# speedup vs baseline: 1.1368x; 1.0003x over previous
"""Sequential-nn4 bf16 kernel for DLLinearZeroDiagonal (y = x @ W.T + bias).

Sharding: RO=4 over output dim x RB=2 over batch.  Per core OC=1024
(NW=8 o-blocks), BC=2048 (NN=4 moving 512-tiles per weight tile).

o-blocks are processed one at a time with the measured-fastest PE
structure (per t: one weight tile + 4 consecutive matmuls; 4 PSUM banks
per accumulation group rotating through an 8-slot pool).  x lives in 32
per-t tiles so the first o-block's matmuls pace with x-DMA arrival
instead of waiting for the whole 16 MB shard.
"""

import numpy as np

N = 4096
RO, RB = 4, 2
OC = N // RO        # 1024
BC = N // RB        # 2048
NW = OC // 128      # 8
NT = N // 128       # 32
NN = BC // 512      # 4

_PROGRAM = None


def _np_bf16():
    import ml_dtypes

    return np.dtype(ml_dtypes.bfloat16)


def _build_program(reps=None):
    import concourse.bacc as bacc
    import concourse.bass as bass
    import concourse.tile as tile
    from concourse import mybir
    from contextlib import ExitStack, nullcontext

    F32 = mybir.dt.float32
    BF16 = mybir.dt.bfloat16

    nc = bacc.Bacc("TRN2", target_bir_lowering=False, debug=False)
    # host-tiled layouts (see _shard_inputs):
    #   xt[t, p, b]         = x[b0+b, 128t+p]
    #   wt[w, p, t*128+o']  = W[o0+128w+o', 128t+p]
    #   bias2[p, w]         = bias[o0+128w+p]
    #   yt[w, o', b]        = y[b0+b, o0+128w+o']
    xt = nc.dram_tensor("xt", [NT, 128, BC], BF16, kind="ExternalInput")
    wt = nc.dram_tensor("wt", [NW, 128, NT * 128], BF16, kind="ExternalInput")
    bias = nc.dram_tensor("bias", [128, NW], F32, kind="ExternalInput")
    yt = nc.dram_tensor("yt", [NW, 128, BC], BF16, kind="ExternalOutput")

    with tile.TileContext(nc) as tc, ExitStack() as ctx:
        xtp = ctx.enter_context(tc.tile_pool(name="xtp", bufs=1))
        wtp = ctx.enter_context(tc.tile_pool(name="wtp", bufs=2))
        bp = ctx.enter_context(tc.tile_pool(name="bp", bufs=1))
        op = ctx.enter_context(tc.tile_pool(name="op", bufs=2))
        pp = ctx.enter_context(tc.tile_pool(name="pp", bufs=8, space="PSUM"))

        loop = tc.For_i(0, reps, 1) if reps is not None else nullcontext()
        with loop:
            bias_sb = bp.tile([128, NW], F32)
            nc.sync.dma_start(bias_sb[:], bass.AP(bias, 0, [[NW, 128], [1, NW]]))
            xr = []
            for t in range(NT):
                x_t = xtp.tile([128, BC], BF16, name=f"xr{t}", tag=f"xr{t}")
                nc.scalar.dma_start(
                    x_t[:], bass.AP(xt, t * 128 * BC, [[BC, 128], [1, BC]])
                )
                xr.append(x_t)

            for w in range(NW):
                slab = wtp.tile([128, NT * 128], BF16)
                nc.sync.dma_start(
                    slab[:],
                    bass.AP(wt, w * 128 * NT * 128,
                            [[NT * 128, 128], [1, NT * 128]]),
                )
                psums = [pp.tile([128, 512], F32, name=f"ps{n}", tag="ps")
                         for n in range(NN)]
                for t in range(NT):
                    lhsT = slab[:, t * 128:(t + 1) * 128]
                    for n in range(NN):
                        nc.tensor.matmul(
                            psums[n][:],
                            lhsT,
                            xr[t][:, n * 512:(n + 1) * 512],
                            start=(t == 0),
                            stop=(t == NT - 1),
                        )
                ot = op.tile([128, BC], BF16)
                for n in range(NN):
                    nc.vector.tensor_scalar_add(
                        ot[:, n * 512:(n + 1) * 512], psums[n][:],
                        bias_sb[:, w:w + 1])
                # y-out on the sync ring: the scalar ring carries only x
                # loads, so the next iteration's x DMAs aren't FIFO-queued
                # behind this iteration's outputs.
                nc.sync.dma_start(
                    bass.AP(yt, w * 128 * BC, [[BC, 128], [1, BC]]),
                    ot[:],
                )
    nc.compile()
    return nc


def _get_program():
    global _PROGRAM
    if _PROGRAM is None:
        _PROGRAM = _build_program()
    return _PROGRAM


def _reconstruct_wt(upper_w: np.ndarray, lower_w: np.ndarray) -> np.ndarray:
    """Dense W [o, j] from the packed strict triangles (row-major fill)."""
    W = np.zeros((N, N), dtype=np.float32)
    iu = np.triu_indices(N, k=1)
    il = np.tril_indices(N, k=-1)
    W[iu] = upper_w
    W[il] = lower_w
    return W


def _shard_inputs(x, upper_w, lower_w, bias):
    bf16 = _np_bf16()
    x = np.asarray(x, dtype=np.float32)
    upper_w = np.asarray(upper_w, dtype=np.float32)
    lower_w = np.asarray(lower_w, dtype=np.float32)
    bias = np.asarray(bias, dtype=np.float32)

    W = _reconstruct_wt(upper_w, lower_w)

    wt_shards = []
    bias_shards = []
    for ob in range(RO):
        Ws = W[ob * OC:(ob + 1) * OC, :]                       # [OC o, N j]
        # wt[w, p, t, o'] = Ws[128w+o', 128t+p]
        wtile = (
            Ws.T.reshape(NT, 128, NW, 128)                     # [t, p, w, o']
            .transpose(2, 1, 0, 3)                             # [w, p, t, o']
            .reshape(NW, 128, NT * 128)
        )
        wt_shards.append(np.ascontiguousarray(wtile.astype(bf16)))
        bias_shards.append(
            np.ascontiguousarray(bias[ob * OC:(ob + 1) * OC].reshape(NW, 128).T)
        )

    xt_shards = []
    for bb in range(RB):
        xs = x[bb * BC:(bb + 1) * BC, :]                       # [BC b, N j]
        xt_shards.append(
            np.ascontiguousarray(xs.T.reshape(NT, 128, BC).astype(bf16))
        )

    in_maps = []
    for c in range(8):
        ob, bb = c // RB, c % RB
        in_maps.append({
            "xt": xt_shards[bb],
            "wt": wt_shards[ob],
            "bias": bias_shards[ob],
        })
    return in_maps


def _assemble(results) -> np.ndarray:
    y = np.empty((N, N), dtype=np.float32)
    for c in range(8):
        ob, bb = c // RB, c % RB
        yt = results[c]["yt"].astype(np.float32)               # [NW, 128, BC]
        y[bb * BC:(bb + 1) * BC, ob * OC:(ob + 1) * OC] = (
            yt.transpose(2, 0, 1).reshape(BC, OC)
        )
    return y


def kernel(x, upper_w, lower_w, bias):
    from concourse import bass_utils

    nc = _get_program()
    in_maps = _shard_inputs(x, upper_w, lower_w, bias)
    res = bass_utils.run_bass_kernel_spmd(nc, in_maps, core_ids=list(range(8)))
    return _assemble(res.results)
